# revision 15
# baseline (speedup 1.0000x reference)
"""Grouped-Query Attention (B=1, L=4096, D=1024, 16 q-heads, 4 kv-heads, hd=64)
on 8 Trainium2 NeuronCores.

Sharding: core c owns q-heads {2c, 2c+1} and their shared kv-head c//2.
Each core computes Q/K/V projections for its heads from the full (replicated)
x, runs dense softmax attention for its 2 heads, and produces a partial
output projection  attn_heads @ Wo[head_rows]  of full shape [4096, 1024]
in bf16. Host sums the 8 partials (fp32) and adds bo.

Key engine-level structure (per core):
  - K^T projection col-packed two sequence-blocks at a time (tile_position
    col groups; separate PSUM banks per accumulation chain). V projection
    x-tile-stationary into natural [k, hd] layout, ones column appended for
    the softmax denominator (AV matmul stationary [V | 1], M=65).
  - Scores: k-tiles alternate PE row groups 0-63 / 64-127 (contraction is
    hd=64) so one group's LDWEIGHTS overlaps the other group's streaming;
    the two q-heads share each K^T stationary. Q^T is kept in both stack
    orders ([Q0;Q1] and [Q1;Q0], swapped by a GpSimd SBUF->SBUF DMA) so
    each head's moving operand exists on both partition halves.
  - Scores land in fp32 PSUM scaled by 1/16: exp runs per (group, head),
    head 0 on ScalarE (activation Exp, scale=16), head 1 mostly on VectorE
    via the custom 8-stage DVE op EXP16_ANT (minimax quadratic ^16); AV
    matmuls run one exp-group behind the score matmuls.
  - Softmax denominators ride the AV matmul (ones column, PSUM row 64);
    normalization via bf16 rank-1 PE broadcast + RECIPROCAL_APPROX_FAST.
  - Out-projection: single C=128 matmul per tile (both heads), head-1
    attention rows stacked onto partitions 64-127 by an SBUF->SBUF DMA;
    chunks spread through the next q-block's k-loop on a dedicated PSUM tag.
"""

import os

os.environ.setdefault("MYCRO_LOCAL_CACHE", "1")

import numpy as np
import ml_dtypes

import concourse.bass as bass
import concourse.bacc as bacc
import concourse.mybir as mybir
from concourse.tile import TileContext
from concourse.bass_utils import run_bass_kernel_spmd

# ---- custom DVE op: EXP16_ANT -------------------------------------------
# out = (c0 + y*(c1 + y*c2))^16 ~= exp(16*y); minimax quadratic fit of e^y
# on |y| <= 0.16 followed by 4 squarings -- exactly the 8 ALU stages of the
# v3 DVE pipeline. Registered via the documented extension path (append to
# concourse.dve_ops.OPS); the per-NEFF uop table is generated at compile.
import concourse.dve_ops as _dve_ops
from concourse.dve_ops import DveOp as _DveOp, OPS as _OPS
from concourse.dve_spec import (
    C0 as _C0, C1 as _C1, C2 as _C2, Spec as _Spec, Src0 as _Src0,
    sq as _sq, lower as _spec_lower,
)
from concourse.dve_uop import DveOpSpec as _DveOpSpec

EXP_C0 = 1.000020401766253
EXP_C1 = 1.0031922899642334
EXP_C2 = 0.4991996999298621


def _exp16_ref(in0, in1, s0, s1, imm2):
    p = (s0 + in0.astype(np.float32) * (s1 + in0.astype(np.float32) * imm2)
         ).astype(np.float32)
    for _ in range(4):
        p = (p * p).astype(np.float32)
    return p


def _register_exp16() -> "_DveOp":
    name = "EXP16_ANT"
    for op in _OPS:
        if op.name == name:
            return op
    spec = _Spec(
        body=_sq(_sq(_sq(_sq(_C0 + _Src0 * (_C1 + _Src0 * _C2))))),
        reference=_exp16_ref,
    )
    shas = {}
    for ver in ("v3", "v4"):
        try:
            shas[ver] = _DveOpSpec(
                name=name, opcode=0, uops=_spec_lower(spec, ver=ver),
                rd1_en=False).sha(ver)
        except Exception:
            pass
    op = _DveOp(name, spec, subdim=False, uops_sha=shas)
    _OPS.append(op)
    _dve_ops.CUSTOM_DVE_SPECS[name] = spec
    _dve_ops._SUB_OPCODE_FOR_NAME[name] = (
        _dve_ops._CUSTOM_DVE_ROW_BASE + len(_OPS) - 1)
    assert _dve_ops._SUB_OPCODE_FOR_NAME[name] < 0x20
    return op


EXP16_ANT = _register_exp16()
# --------------------------------------------------------------------------

BF16 = mybir.dt.bfloat16
F32 = mybir.dt.float32
AF = mybir.ActivationFunctionType

D = 1024
L = 4096
NHEAD = 16
NKV = 4
HD = 64
NCORES = 8
HPC = 2                # q heads per core
QB = 512               # q-block width
NQB = L // QB          # 8
KT = 128               # k-tile
NKT = L // KT          # 32
KG = 2                 # k-tiles per exp group
NF = D // 128          # 8 feature chunks
SCALE = 1.0 / (8.0 * 16.0)   # 1/sqrt(64) folded with the exp 1/16 pre-scale
# head-1 exp engine per group index: D=VectorE custom op, A=ScalarE
# (head-0 exp always runs on ScalarE)
EXP_PATTERN = "DDDADDDA"

_CACHE = {}


def _build(has_bias):
    nc = bacc.Bacc("TRN2", target_bir_lowering=False, debug=False)

    # folded layouts: dim0 = sbuf partition, dim1 = f-chunk * inner
    xT = nc.declare_dram_parameter("xT", [128, NF * L], BF16, isOutput=False)
    wq = nc.declare_dram_parameter("wq", [128, NF * HPC * HD], BF16,
                                   isOutput=False)
    wk = nc.declare_dram_parameter("wk", [128, NF * HD], BF16, isOutput=False)
    wv = nc.declare_dram_parameter("wv", [128, NF * HD], BF16, isOutput=False)
    wo2 = nc.declare_dram_parameter("wo2", [HPC * HD, D], BF16, isOutput=False)
    bq = nc.declare_dram_parameter("bq", [1, HPC * HD], BF16, isOutput=False)
    bk = nc.declare_dram_parameter("bk", [1, HD], BF16, isOutput=False)
    bv = nc.declare_dram_parameter("bv", [1, HD], BF16, isOutput=False)
    out = nc.declare_dram_parameter("out", [L, D], BF16, isOutput=True)

    with TileContext(nc) as tc:
        with (
            tc.tile_pool(name="sing", bufs=1) as sing,
            tc.tile_pool(name="ptp", bufs=5) as ptp,
            tc.tile_pool(name="attp", bufs=2) as attp,
            tc.tile_pool(name="nrm", bufs=2) as nrm,
            tc.tile_pool(name="obp", bufs=3) as obp,
            tc.tile_pool(name="psSt", bufs=2, space="PSUM") as psSt,
            tc.tile_pool(name="psAv", bufs=2, space="PSUM") as psAv,
        ):
            # ---- resident SBUF tensors ----
            xT_sb = sing.tile([128, NF, L], BF16)
            wq_sb = sing.tile([128, NF, HPC * HD], BF16)
            wk_sb = sing.tile([128, NF, HD], BF16)
            wv_sb = sing.tile([128, NF, HD], BF16)
            wo2_sb = sing.tile([128, D], BF16)
            KT2 = sing.tile([128, L], BF16)       # K^T duplicated on both halves
            QT2 = sing.tile([128, L], BF16)       # [Q0^T; Q1^T] * SCALE
            QT2s = sing.tile([128, L], BF16)      # swapped: [Q1^T; Q0^T]
            V_sb = sing.tile([128, NKT, HD + 1], BF16)  # col 64 = 1.0
            ones_f = sing.tile([HD + 1, HD], BF16)     # recip bcast stationary
            if has_bias:
                bq_sb = sing.tile([1, HPC * HD], BF16)
                bk_sb = sing.tile([1, HD], BF16)
                bv_sb = sing.tile([1, HD], BF16)
                ones_b = sing.tile([1, QB], BF16)

            # ---- input DMAs. All host tensors are pre-folded to
            # partition-major [128, f, *] layouts so each load is a single
            # strided-AP dma_start: the sequencers issue ~2us per
            # descriptor, so descriptor COUNT (not bytes) was the startup
            # bottleneck. First the K-projection's working set (wk + the
            # first xT n-block in two halves, split across queues), then
            # the rest.
            nc.sync.dma_start(out=wk_sb[:, :, :], in_=wk[:, :])
            engs = [nc.sync, nc.scalar, nc.gpsimd]
            for b in range(NQB):
                ns = slice(QB * b, QB * (b + 1))
                nsf = slice(NF * QB * b, NF * QB * (b + 1))
                engs[b % 3 if b >= 2 else b].dma_start(out=xT_sb[:, :, ns],
                                                       in_=xT[:, nsf])
            nc.gpsimd.dma_start(out=wv_sb[:, :, :], in_=wv[:, :])
            nc.gpsimd.dma_start(out=wq_sb[:, :, :], in_=wq[:, :])
            if has_bias:
                nc.gpsimd.dma_start(out=bq_sb, in_=bq[:, :])
                nc.gpsimd.dma_start(out=bk_sb, in_=bk[:, :])
                nc.gpsimd.dma_start(out=bv_sb, in_=bv[:, :])
                nc.gpsimd.memset(ones_b, 1.0)
            nc.gpsimd.dma_start(out=wo2_sb, in_=wo2[:, :])
            nc.gpsimd.memset(ones_f, 1.0)
            nc.gpsimd.memset(V_sb[:, :, HD], 1.0)

            # ---- K^T projection, col-packed pairs of n-blocks ----
            # Chain A (cols 0-63 -> psum partitions 0-63, bank 0) and chain B
            # (tile_position (0,64) -> partitions 64-127, bank 1). Separate
            # banks because start=True clears has_written bank-wide.
            for w_sb, dst, nm in ((wk_sb, KT2, "kps"),):
                b_sb = bk_sb if has_bias else None
                for p in range(NQB // 2):
                    nsA = slice(QB * 2 * p, QB * (2 * p + 1))
                    nsB = slice(QB * (2 * p + 1), QB * (2 * p + 2))
                    pp = psSt.tile([128, 2, QB], F32, tag="st", name=nm)
                    for f in range(NF):
                        last = f == NF - 1 and not has_bias
                        nc.tensor.matmul(pp[0:HD, 0, :], w_sb[:, f, :],
                                         xT_sb[:, f, nsA], start=(f == 0),
                                         stop=last, tile_position=(0, 0))
                        nc.tensor.matmul(pp[HD:128, 1, :], w_sb[:, f, :],
                                         xT_sb[:, f, nsB], start=(f == 0),
                                         stop=last, tile_position=(0, 64))
                    if has_bias:
                        nc.tensor.matmul(pp[0:HD, 0, :], b_sb, ones_b,
                                         start=False, stop=True,
                                         tile_position=(0, 0))
                        nc.tensor.matmul(pp[HD:128, 1, :], b_sb, ones_b,
                                         start=False, stop=True,
                                         tile_position=(0, 64))
                    nc.vector.tensor_copy(dst[0:HD, nsA], pp[0:HD, 0, :])
                    nc.vector.tensor_copy(dst[HD:128, nsB], pp[HD:128, 1, :])

            # V projection: x-tile stationary, natural [k, hd] layout
            for lt in range(NKT):
                ls = slice(KT * lt, KT * (lt + 1))
                vps = psSt.tile([128, HD], F32, tag="st", name="vps")
                for f in range(NF):
                    nc.tensor.matmul(vps, xT_sb[:, f, ls], wv_sb[:, f, :],
                                     start=(f == 0),
                                     stop=(not has_bias and f == NF - 1))
                if has_bias:
                    nc.tensor.matmul(vps, ones_b[:, 0:KT], bv_sb,
                                     start=False, stop=True)
                nc.vector.tensor_copy(V_sb[:, lt, 0:HD], vps)

            # KT2 dup fixups: SBUF->SBUF DMA so both halves hold full K^T.
            # Issued from the Scalar queue so the (busy) Sync queue can't
            # head-of-line block them.
            for p in range(NQB // 2):
                nsA = slice(QB * 2 * p, QB * (2 * p + 1))
                nsB = slice(QB * (2 * p + 1), QB * (2 * p + 2))
                nc.scalar.dma_start(out=KT2[HD:128, nsA], in_=KT2[0:HD, nsA])
                nc.scalar.dma_start(out=KT2[0:HD, nsB], in_=KT2[HD:128, nsB])

            # ---- q-block loop (out-proj of q-1 spread inside q's k-loop) ----
            pending = None

            def emit_outproj_chunk(q, atT2, lc, tail=False):
                lcs = slice(128 * lc, 128 * (lc + 1))
                if tail:
                    # the scores 'st' slots are idle at the tail; using them
                    # double-buffers the final chunks (the single 'op' slot
                    # would serialize each chunk behind the previous DMA)
                    ops = psSt.tile([128, 2, QB], F32, tag="st", name="ops_t")
                else:
                    ops = psSt.tile([128, 2, QB], F32, tag="op", bufs=1,
                                    name="ops")
                for n in range(2):
                    ns = slice(QB * n, QB * (n + 1))
                    mm = nc.tensor.matmul(ops[:, n, :], atT2[:, lcs],
                                          wo2_sb[:, ns], start=True, stop=True)
                    if n == 1:
                        mm.ins.ldweights = False
                osb = obp.tile([128, D], BF16, tag="ob", name="osb")
                if lc % 2 == 0:
                    nc.vector.tensor_copy(osb, ops)
                else:
                    nc.scalar.copy(osb, ops)
                nc.sync.dma_start(
                    out=out[QB * q + 128 * lc:QB * q + 128 * (lc + 1), :],
                    in_=osb)

            def emit_qproj(q):
                # Q^T projection: M=128 -> [Q0^T; Q1^T] stacked, then scale
                # (scale-copy on ScalarE so the DVE exp queue can't delay it)
                qs = slice(QB * q, QB * (q + 1))
                qps = psSt.tile([128, QB], F32, tag="st", name="qps")
                for f in range(NF):
                    nc.tensor.matmul(qps, wq_sb[:, f, :], xT_sb[:, f, qs],
                                     start=(f == 0),
                                     stop=(not has_bias and f == NF - 1))
                if has_bias:
                    nc.tensor.matmul(qps, bq_sb, ones_b, start=False, stop=True)
                nc.scalar.activation(QT2[:, qs], qps, AF.Copy, scale=SCALE)
                nc.gpsimd.dma_start(out=QT2s[HD:128, qs], in_=QT2[0:HD, qs])
                nc.gpsimd.dma_start(out=QT2s[0:HD, qs], in_=QT2[HD:128, qs])

            emit_qproj(0)
            for q in range(NQB):
                qs = slice(QB * q, QB * (q + 1))
                avps = [psAv.tile([HD + 1, QB], F32, tag="av", name=f"avps{h}")
                        for h in range(HPC)]

                def emit_av(k0, gs, ptsb):
                    for j in range(gs):
                        for h in range(HPC):
                            mm = nc.tensor.matmul(avps[h], V_sb[:, k0 + j, :],
                                                  ptsb[h][:, j, :],
                                                  start=(k0 + j == 0),
                                                  stop=(k0 + j == NKT - 1))
                            if h == 1:
                                mm.ins.ldweights = False

                prevs = []  # AV runs two exp-groups behind the scores
                k = 0
                g = 0
                while k < NKT:
                    gs = min(KG, NKT - k)
                    stps = [psSt.tile([128, KG, QB], F32, tag="st",
                                      name=f"stps{h}") for h in range(HPC)]
                    ptsb = [ptp.tile([128, KG, QB], BF16, tag=f"pt{h}",
                                     name=f"ptsb{h}") for h in range(HPC)]
                    for j in range(gs):
                        kt = k + j
                        ks = slice(KT * kt, KT * (kt + 1))
                        # k-tiles alternate PE row groups; the two heads share
                        # the K^T stationary (second matmul skips LDWEIGHTS),
                        # so group A streams while group B loads and vice
                        # versa. Q0/Q1 come from the natural and swapped
                        # stacks so each lands on the right partitions.
                        if kt % 2 == 0:
                            m0 = nc.tensor.matmul(
                                stps[0][:, j, :], KT2[0:HD, ks],
                                QT2[0:HD, qs], start=True, stop=True)
                            m1 = nc.tensor.matmul(
                                stps[1][:, j, :], KT2[0:HD, ks],
                                QT2s[0:HD, qs], start=True, stop=True)
                        else:
                            m0 = nc.tensor.matmul(
                                stps[0][:, j, :], KT2[HD:128, ks],
                                QT2s[HD:128, qs], start=True, stop=True)
                            m1 = nc.tensor.matmul(
                                stps[1][:, j, :], KT2[HD:128, ks],
                                QT2[HD:128, qs], start=True, stop=True)
                        m1.ins.ldweights = False
                    nc.scalar.activation(ptsb[0][:, 0:gs, :],
                                         stps[0][:, 0:gs, :], AF.Exp,
                                         scale=16.0)
                    if EXP_PATTERN[g % len(EXP_PATTERN)] == "D":
                        nc.vector._custom_dve(
                            EXP16_ANT, out=ptsb[1][:, 0:gs, :],
                            in0=stps[1][:, 0:gs, :],
                            s0=EXP_C0, s1=EXP_C1, imm2=EXP_C2)
                    else:
                        nc.scalar.activation(ptsb[1][:, 0:gs, :],
                                             stps[1][:, 0:gs, :], AF.Exp,
                                             scale=16.0)
                    if len(prevs) >= 2:
                        emit_av(*prevs.pop(0))
                    prevs.append((k, gs, ptsb))
                    k += gs
                    g += 1
                    if pending is not None and g % 2 == 0 and 4 <= g <= 10:
                        emit_outproj_chunk(*pending, g // 2 - 2)
                        if g == 10:
                            pending = None
                    if g == 9 and q + 1 < NQB:
                        # hoisted Q projection: the qps->scale->swap-DMA chain
                        # completes during groups 10-15 so the next q-block's
                        # scores are never blocked on Q^T availability.
                        emit_qproj(q + 1)
                for pv in prevs:
                    emit_av(*pv)

                # epilogue: free avps ASAP (early PSUM->SBUF f32 copies, one
                # per engine) so the next q-block's AV chain isn't blocked on
                # the normalization chain; rank-1 denominator broadcast into
                # an 'op'-tagged PSUM slot so the 'st' rotation (scores) is
                # never blocked on the reciprocal chain.
                last = q == NQB - 1
                atT2 = attp.tile([128, QB], BF16, tag="at", name="atT2")
                dsbs = []
                avus = []
                for h in range(HPC):
                    dsb = nrm.tile([HD + 1, QB], BF16, tag=f"rs{h}", name="dsb")
                    if last and h == 0:
                        nc.scalar.copy(dsb[HD:HD + 1, :],
                                       avps[h][HD:HD + 1, :])
                    else:
                        nc.vector.tensor_copy(dsb[HD:HD + 1, :],
                                              avps[h][HD:HD + 1, :])
                    dsbs.append(dsb)
                    if last:
                        # no next q-block to unblock: read avps directly and
                        # skip the extra copy hop to shorten the tail chain
                        avus.append(avps[h][0:HD, :])
                        continue
                    avu = attp.tile([HD, QB], F32, tag=f"au{h}", name="avu")
                    if h == 0:
                        nc.scalar.copy(avu, avps[h][0:HD, :])
                    else:
                        nc.vector.tensor_copy(avu, avps[h][0:HD, :])
                    avus.append(avu)
                for h in range(HPC):
                    rbps = psSt.tile([HD, QB], F32, tag="op", bufs=1,
                                     name="rbps")
                    nc.tensor.matmul(rbps, ones_f[HD:HD + 1, :],
                                     dsbs[h][HD:HD + 1, :], start=True,
                                     stop=True)
                    rrb = nrm.tile([HD, QB], F32, tag="rr", name="rrb")
                    nc.vector.reciprocal_approx_fast(out=rrb, in_=rbps)
                    if h == 0:
                        nc.vector.tensor_mul(atT2[0:HD, :], avus[h], rrb)
                    else:
                        atmp = attp.tile([HD, QB], BF16, tag="atm", name="atmp")
                        nc.vector.tensor_mul(atmp, avus[h], rrb)
                        nc.sync.dma_start(out=atT2[HD:128, :], in_=atmp)
                pending = (q, atT2)
            for lc in range(QB // 128):
                emit_outproj_chunk(*pending, lc, tail=True)
    nc.finalize()
    return nc


def _fold(M):
    # [1024, X] -> [128, 8, X]: row 128f+p lands at [p, f, :]
    X = M.shape[1]
    return np.ascontiguousarray(
        M.reshape(8, 128, X).transpose(1, 0, 2).reshape(128, 8 * X))


def _fold_blocks(M, nb):
    # [1024, L] -> [128, nb, 8, L//nb]: element (128f+p, B*b+j) lands at
    # [p, b, f, j] so each n-block load is one contiguous DRAM range.
    X = M.shape[1]
    B = X // nb
    return np.ascontiguousarray(
        M.reshape(8, 128, nb, B).transpose(1, 2, 0, 3).reshape(128, 8 * X))


def _prep_inputs(x, Wq, bq, Wk, bk, Wv, bv, Wo, bo):
    bf = ml_dtypes.bfloat16
    xT = np.ascontiguousarray(np.asarray(x, dtype=np.float32)[0].T).astype(bf)
    Wq = np.asarray(Wq, dtype=np.float32)
    Wk = np.asarray(Wk, dtype=np.float32)
    Wv = np.asarray(Wv, dtype=np.float32)
    Wo = np.asarray(Wo, dtype=np.float32)
    bq = np.asarray(bq, dtype=np.float32)
    bk = np.asarray(bk, dtype=np.float32)
    bv = np.asarray(bv, dtype=np.float32)
    has_bias = bool(np.any(bq) or np.any(bk) or np.any(bv))
    xT_f = _fold_blocks(xT, 8)
    in_maps = []
    for c in range(NCORES):
        qsl = slice(HPC * HD * c, HPC * HD * (c + 1))
        kv = c // 2
        ksl = slice(HD * kv, HD * (kv + 1))
        in_maps.append({
            "xT": xT_f,
            "wq": _fold(np.ascontiguousarray(Wq[:, qsl]).astype(bf)),
            "wk": _fold(np.ascontiguousarray(Wk[:, ksl]).astype(bf)),
            "wv": _fold(np.ascontiguousarray(Wv[:, ksl]).astype(bf)),
            "wo2": np.ascontiguousarray(
                Wo[HPC * HD * c:HPC * HD * (c + 1), :]).astype(bf),
            "bq": bq[qsl].reshape(1, -1).astype(bf),
            "bk": bk[ksl].reshape(1, -1).astype(bf),
            "bv": bv[ksl].reshape(1, -1).astype(bf),
        })
    return in_maps, has_bias


def run(inputs, trace=False):
    in_maps, has_bias = _prep_inputs(**inputs)
    key = ("nc", has_bias)
    if key not in _CACHE:
        _CACHE[key] = _build(has_bias)
    nc = _CACHE[key]
    res = run_bass_kernel_spmd(nc, in_maps, list(range(NCORES)), trace=trace)
    bo = np.asarray(inputs["bo"], dtype=np.float32)
    acc = np.zeros((L, D), dtype=np.float32)
    for r in res.results:
        acc += np.asarray(r["out"], dtype=np.float32)
    out = (acc + bo).reshape(1, L, D)
    return out, res


def kernel(**inputs):
    out, _ = run(inputs, trace=False)
    return out



# revision 17
# speedup vs baseline: 1.1813x; 1.1813x over previous
"""Grouped-Query Attention (B=1, L=4096, D=1024, 16 q-heads, 4 kv-heads, hd=64)
on 8 Trainium2 NeuronCores.

Sharding: core c owns q-heads {2c, 2c+1} and their shared kv-head c//2.
Each core computes Q/K/V projections for its heads from the full (replicated)
x, runs dense softmax attention for its 2 heads, and produces a partial
output projection  attn_heads @ Wo[head_rows]  of full shape [4096, 1024]
in bf16. Host sums the 8 partials (fp32) and adds bo.

Key engine-level structure (per core):
  - K^T projection col-packed two sequence-blocks at a time (tile_position
    col groups; separate PSUM banks per accumulation chain). V projection
    x-tile-stationary into natural [k, hd] layout, ones column appended for
    the softmax denominator (AV matmul stationary [V | 1], M=65).
  - Scores: k-tiles alternate PE row groups 0-63 / 64-127 (contraction is
    hd=64) so one group's LDWEIGHTS overlaps the other group's streaming;
    the two q-heads share each K^T stationary. Q^T is kept in both stack
    orders ([Q0;Q1] and [Q1;Q0], swapped by a GpSimd SBUF->SBUF DMA) so
    each head's moving operand exists on both partition halves.
  - Scores land in fp32 PSUM scaled by 1/16: exp runs per (group, head),
    head 0 on ScalarE (activation Exp, scale=16), head 1 mostly on VectorE
    via the custom 8-stage DVE op EXP16_ANT (minimax quadratic ^16); AV
    matmuls run one exp-group behind the score matmuls.
  - Softmax denominators ride the AV matmul (ones column, PSUM row 64);
    normalization via bf16 rank-1 PE broadcast + RECIPROCAL_APPROX_FAST.
  - Out-projection: single C=128 matmul per tile (both heads), head-1
    attention rows stacked onto partitions 64-127 by an SBUF->SBUF DMA;
    chunks spread through the next q-block's k-loop on a dedicated PSUM tag.
"""

import os

os.environ.setdefault("MYCRO_LOCAL_CACHE", "1")

import numpy as np
import ml_dtypes

import concourse.bass as bass
import concourse.bacc as bacc
import concourse.mybir as mybir
from concourse.tile import TileContext
from concourse.bass_utils import run_bass_kernel_spmd

# ---- custom DVE op: EXP16_ANT -------------------------------------------
# out = (c0 + y*(c1 + y*c2))^16 ~= exp(16*y); minimax quadratic fit of e^y
# on |y| <= 0.16 followed by 4 squarings -- exactly the 8 ALU stages of the
# v3 DVE pipeline. Registered via the documented extension path (append to
# concourse.dve_ops.OPS); the per-NEFF uop table is generated at compile.
import concourse.dve_ops as _dve_ops
from concourse.dve_ops import DveOp as _DveOp, OPS as _OPS
from concourse.dve_spec import (
    C0 as _C0, C1 as _C1, C2 as _C2, Spec as _Spec, Src0 as _Src0,
    sq as _sq, lower as _spec_lower,
)
from concourse.dve_uop import DveOpSpec as _DveOpSpec

EXP_C0 = 1.000020401766253
EXP_C1 = 1.0031922899642334
EXP_C2 = 0.4991996999298621


def _exp16_ref(in0, in1, s0, s1, imm2):
    p = (s0 + in0.astype(np.float32) * (s1 + in0.astype(np.float32) * imm2)
         ).astype(np.float32)
    for _ in range(4):
        p = (p * p).astype(np.float32)
    return p


def _register_exp16() -> "_DveOp":
    name = "EXP16_ANT"
    for op in _OPS:
        if op.name == name:
            return op
    spec = _Spec(
        body=_sq(_sq(_sq(_sq(_C0 + _Src0 * (_C1 + _Src0 * _C2))))),
        reference=_exp16_ref,
    )
    shas = {}
    for ver in ("v3", "v4"):
        try:
            shas[ver] = _DveOpSpec(
                name=name, opcode=0, uops=_spec_lower(spec, ver=ver),
                rd1_en=False).sha(ver)
        except Exception:
            pass
    op = _DveOp(name, spec, subdim=False, uops_sha=shas)
    _OPS.append(op)
    _dve_ops.CUSTOM_DVE_SPECS[name] = spec
    _dve_ops._SUB_OPCODE_FOR_NAME[name] = (
        _dve_ops._CUSTOM_DVE_ROW_BASE + len(_OPS) - 1)
    assert _dve_ops._SUB_OPCODE_FOR_NAME[name] < 0x20
    return op


EXP16_ANT = _register_exp16()
# --------------------------------------------------------------------------

BF16 = mybir.dt.bfloat16
F32 = mybir.dt.float32
AF = mybir.ActivationFunctionType

D = 1024
L = 4096
NHEAD = 16
NKV = 4
HD = 64
NCORES = 8
HPC = 2                # q heads per core
QB = 512               # q-block width
NQB = L // QB          # 8
KT = 128               # k-tile
NKT = L // KT          # 32
KG = 2                 # k-tiles per exp group
NF = D // 128          # 8 feature chunks
SCALE = 1.0 / (8.0 * 16.0)   # 1/sqrt(64) folded with the exp 1/16 pre-scale
# head-1 exp engine per group index: D=VectorE custom op, A=ScalarE
# (head-0 exp always runs on ScalarE)
EXP_PATTERN = "DDDADDDA"

_CACHE = {}


def _build(has_bias):
    nc = bacc.Bacc("TRN2", target_bir_lowering=False, debug=False)

    # folded layouts: dim0 = sbuf partition, dim1 = f-chunk * inner
    xT = nc.declare_dram_parameter("xT", [128, NF * L], BF16, isOutput=False)
    wq = nc.declare_dram_parameter("wq", [128, NF * HPC * HD], BF16,
                                   isOutput=False)
    wk = nc.declare_dram_parameter("wk", [128, NF * HD], BF16, isOutput=False)
    wv = nc.declare_dram_parameter("wv", [128, NF * HD], BF16, isOutput=False)
    wo2 = nc.declare_dram_parameter("wo2", [HPC * HD, D], BF16, isOutput=False)
    bq = nc.declare_dram_parameter("bq", [1, HPC * HD], BF16, isOutput=False)
    bk = nc.declare_dram_parameter("bk", [1, HD], BF16, isOutput=False)
    bv = nc.declare_dram_parameter("bv", [1, HD], BF16, isOutput=False)
    out = nc.declare_dram_parameter("out", [L, D], BF16, isOutput=True)

    with TileContext(nc) as tc:
        with (
            tc.tile_pool(name="sing", bufs=1) as sing,
            tc.tile_pool(name="ptp", bufs=5) as ptp,
            tc.tile_pool(name="attp", bufs=2) as attp,
            tc.tile_pool(name="nrm", bufs=2) as nrm,
            tc.tile_pool(name="obp", bufs=3) as obp,
            tc.tile_pool(name="psSt", bufs=2, space="PSUM") as psSt,
            tc.tile_pool(name="psAv", bufs=2, space="PSUM") as psAv,
        ):
            # ---- resident SBUF tensors ----
            xT_sb = sing.tile([128, NF, L], BF16)
            wq_sb = sing.tile([128, NF, HPC * HD], BF16)
            wk_sb = sing.tile([128, NF, HD], BF16)
            wv_sb = sing.tile([128, NF, HD], BF16)
            wo2_sb = sing.tile([128, D], BF16)
            KT2 = sing.tile([128, L], BF16)       # K^T duplicated on both halves
            QT2 = sing.tile([128, L], BF16)       # [Q0^T; Q1^T] * SCALE
            QT2s = sing.tile([128, L], BF16)      # swapped: [Q1^T; Q0^T]
            V_sb = sing.tile([128, NKT, HD + 1], BF16)  # col 64 = 1.0
            ones_f = sing.tile([HD + 1, HD], BF16)     # recip bcast stationary
            if has_bias:
                bq_sb = sing.tile([1, HPC * HD], BF16)
                bk_sb = sing.tile([1, HD], BF16)
                bv_sb = sing.tile([1, HD], BF16)
                ones_b = sing.tile([1, QB], BF16)

            # ---- input DMAs. All host tensors are pre-folded to
            # partition-major [128, f, *] layouts so each load is a single
            # strided-AP dma_start: the sequencers issue ~2us per
            # descriptor, so descriptor COUNT (not bytes) was the startup
            # bottleneck. First the K-projection's working set (wk + the
            # first xT n-block in two halves, split across queues), then
            # the rest.
            nc.sync.dma_start(out=wk_sb[:, :, :], in_=wk[:, :])
            engs = [nc.sync, nc.scalar]
            for b in range(NQB):
                ns = slice(QB * b, QB * (b + 1))
                for hh in range(2):
                    fs = slice(NF // 2 * hh, NF // 2 * (hh + 1))
                    nsf = slice(NF * QB * b + NF // 2 * QB * hh,
                                NF * QB * b + NF // 2 * QB * (hh + 1))
                    engs[(2 * b + hh) % 2].dma_start(out=xT_sb[:, fs, ns],
                                                     in_=xT[:, nsf])
            nc.gpsimd.dma_start(out=wv_sb[:, :, :], in_=wv[:, :])
            nc.gpsimd.dma_start(out=wq_sb[:, :, :], in_=wq[:, :])
            if has_bias:
                nc.gpsimd.dma_start(out=bq_sb, in_=bq[:, :])
                nc.gpsimd.dma_start(out=bk_sb, in_=bk[:, :])
                nc.gpsimd.dma_start(out=bv_sb, in_=bv[:, :])
                nc.gpsimd.memset(ones_b, 1.0)
            nc.gpsimd.dma_start(out=wo2_sb, in_=wo2[:, :])
            nc.gpsimd.memset(ones_f, 1.0)
            nc.gpsimd.memset(V_sb[:, :, HD], 1.0)

            # ---- K^T projection, col-packed pairs of n-blocks ----
            # Chain A (cols 0-63 -> psum partitions 0-63, bank 0) and chain B
            # (tile_position (0,64) -> partitions 64-127, bank 1). Separate
            # banks because start=True clears has_written bank-wide.
            for w_sb, dst, nm in ((wk_sb, KT2, "kps"),):
                b_sb = bk_sb if has_bias else None
                for p in range(NQB // 2):
                    nsA = slice(QB * 2 * p, QB * (2 * p + 1))
                    nsB = slice(QB * (2 * p + 1), QB * (2 * p + 2))
                    pp = psSt.tile([128, 2, QB], F32, tag="st", name=nm)
                    for f in range(NF):
                        last = f == NF - 1 and not has_bias
                        nc.tensor.matmul(pp[0:HD, 0, :], w_sb[:, f, :],
                                         xT_sb[:, f, nsA], start=(f == 0),
                                         stop=last, tile_position=(0, 0))
                        nc.tensor.matmul(pp[HD:128, 1, :], w_sb[:, f, :],
                                         xT_sb[:, f, nsB], start=(f == 0),
                                         stop=last, tile_position=(0, 64))
                    if has_bias:
                        nc.tensor.matmul(pp[0:HD, 0, :], b_sb, ones_b,
                                         start=False, stop=True,
                                         tile_position=(0, 0))
                        nc.tensor.matmul(pp[HD:128, 1, :], b_sb, ones_b,
                                         start=False, stop=True,
                                         tile_position=(0, 64))
                    nc.vector.tensor_copy(dst[0:HD, nsA], pp[0:HD, 0, :])
                    nc.vector.tensor_copy(dst[HD:128, nsB], pp[HD:128, 1, :])

            # V projection: x-tile stationary, natural [k, hd] layout
            for lt in range(NKT):
                ls = slice(KT * lt, KT * (lt + 1))
                vps = psSt.tile([128, HD], F32, tag="st", name="vps")
                for f in range(NF):
                    nc.tensor.matmul(vps, xT_sb[:, f, ls], wv_sb[:, f, :],
                                     start=(f == 0),
                                     stop=(not has_bias and f == NF - 1))
                if has_bias:
                    nc.tensor.matmul(vps, ones_b[:, 0:KT], bv_sb,
                                     start=False, stop=True)
                nc.vector.tensor_copy(V_sb[:, lt, 0:HD], vps)

            # KT2 dup fixups: SBUF->SBUF DMA so both halves hold full K^T.
            # Issued from the Scalar queue so the (busy) Sync queue can't
            # head-of-line block them.
            for p in range(NQB // 2):
                nsA = slice(QB * 2 * p, QB * (2 * p + 1))
                nsB = slice(QB * (2 * p + 1), QB * (2 * p + 2))
                nc.scalar.dma_start(out=KT2[HD:128, nsA], in_=KT2[0:HD, nsA])
                nc.scalar.dma_start(out=KT2[0:HD, nsB], in_=KT2[HD:128, nsB])

            # ---- q-block loop (out-proj of q-1 spread inside q's k-loop) ----
            pending = None

            def emit_outproj_chunk(q, atT2, lc, tail=False):
                lcs = slice(128 * lc, 128 * (lc + 1))
                if tail:
                    # the scores 'st' slots are idle at the tail; using them
                    # double-buffers the final chunks (the single 'op' slot
                    # would serialize each chunk behind the previous DMA)
                    ops = psSt.tile([128, 2, QB], F32, tag="st", name="ops_t")
                else:
                    ops = psSt.tile([128, 2, QB], F32, tag="op", bufs=1,
                                    name="ops")
                for n in range(2):
                    ns = slice(QB * n, QB * (n + 1))
                    mm = nc.tensor.matmul(ops[:, n, :], atT2[:, lcs],
                                          wo2_sb[:, ns], start=True, stop=True)
                    if n == 1:
                        mm.ins.ldweights = False
                osb = obp.tile([128, D], BF16, tag="ob", name="osb")
                if lc % 2 == 0:
                    nc.vector.tensor_copy(osb, ops)
                else:
                    nc.scalar.copy(osb, ops)
                nc.sync.dma_start(
                    out=out[QB * q + 128 * lc:QB * q + 128 * (lc + 1), :],
                    in_=osb)

            def emit_qproj(q):
                # Q^T projection: M=128 -> [Q0^T; Q1^T] stacked, then scale
                # (scale-copy on ScalarE so the DVE exp queue can't delay it)
                qs = slice(QB * q, QB * (q + 1))
                qps = psSt.tile([128, QB], F32, tag="st", name="qps")
                for f in range(NF):
                    nc.tensor.matmul(qps, wq_sb[:, f, :], xT_sb[:, f, qs],
                                     start=(f == 0),
                                     stop=(not has_bias and f == NF - 1))
                if has_bias:
                    nc.tensor.matmul(qps, bq_sb, ones_b, start=False, stop=True)
                nc.scalar.activation(QT2[:, qs], qps, AF.Copy, scale=SCALE)
                nc.gpsimd.dma_start(out=QT2s[HD:128, qs], in_=QT2[0:HD, qs])
                nc.gpsimd.dma_start(out=QT2s[0:HD, qs], in_=QT2[HD:128, qs])

            emit_qproj(0)
            for q in range(NQB):
                qs = slice(QB * q, QB * (q + 1))
                avps = [psAv.tile([HD + 1, QB], F32, tag="av", name=f"avps{h}")
                        for h in range(HPC)]

                def emit_av(k0, gs, ptsb):
                    for j in range(gs):
                        for h in range(HPC):
                            mm = nc.tensor.matmul(avps[h], V_sb[:, k0 + j, :],
                                                  ptsb[h][:, j, :],
                                                  start=(k0 + j == 0),
                                                  stop=(k0 + j == NKT - 1))
                            if h == 1:
                                mm.ins.ldweights = False

                prevs = []  # AV runs two exp-groups behind the scores
                k = 0
                g = 0
                while k < NKT:
                    gs = min(KG, NKT - k)
                    stps = [psSt.tile([128, KG, QB], F32, tag="st",
                                      name=f"stps{h}") for h in range(HPC)]
                    ptsb = [ptp.tile([128, KG, QB], BF16, tag=f"pt{h}",
                                     name=f"ptsb{h}") for h in range(HPC)]
                    for j in range(gs):
                        kt = k + j
                        ks = slice(KT * kt, KT * (kt + 1))
                        # k-tiles alternate PE row groups; the two heads share
                        # the K^T stationary (second matmul skips LDWEIGHTS),
                        # so group A streams while group B loads and vice
                        # versa. Q0/Q1 come from the natural and swapped
                        # stacks so each lands on the right partitions.
                        if kt % 2 == 0:
                            m0 = nc.tensor.matmul(
                                stps[0][:, j, :], KT2[0:HD, ks],
                                QT2[0:HD, qs], start=True, stop=True)
                            m1 = nc.tensor.matmul(
                                stps[1][:, j, :], KT2[0:HD, ks],
                                QT2s[0:HD, qs], start=True, stop=True)
                        else:
                            m0 = nc.tensor.matmul(
                                stps[0][:, j, :], KT2[HD:128, ks],
                                QT2s[HD:128, qs], start=True, stop=True)
                            m1 = nc.tensor.matmul(
                                stps[1][:, j, :], KT2[HD:128, ks],
                                QT2[HD:128, qs], start=True, stop=True)
                        m1.ins.ldweights = False
                    nc.scalar.activation(ptsb[0][:, 0:gs, :],
                                         stps[0][:, 0:gs, :], AF.Exp,
                                         scale=16.0)
                    if EXP_PATTERN[g % len(EXP_PATTERN)] == "D":
                        nc.vector._custom_dve(
                            EXP16_ANT, out=ptsb[1][:, 0:gs, :],
                            in0=stps[1][:, 0:gs, :],
                            s0=EXP_C0, s1=EXP_C1, imm2=EXP_C2)
                    else:
                        nc.scalar.activation(ptsb[1][:, 0:gs, :],
                                             stps[1][:, 0:gs, :], AF.Exp,
                                             scale=16.0)
                    if len(prevs) >= 2:
                        emit_av(*prevs.pop(0))
                    prevs.append((k, gs, ptsb))
                    k += gs
                    g += 1
                    if pending is not None and g % 2 == 0 and 4 <= g <= 10:
                        emit_outproj_chunk(*pending, g // 2 - 2)
                        if g == 10:
                            pending = None
                    if g == 9 and q + 1 < NQB:
                        # hoisted Q projection: the qps->scale->swap-DMA chain
                        # completes during groups 10-15 so the next q-block's
                        # scores are never blocked on Q^T availability.
                        emit_qproj(q + 1)
                for pv in prevs:
                    emit_av(*pv)

                # epilogue: free avps ASAP (early PSUM->SBUF f32 copies, one
                # per engine) so the next q-block's AV chain isn't blocked on
                # the normalization chain; rank-1 denominator broadcast into
                # an 'op'-tagged PSUM slot so the 'st' rotation (scores) is
                # never blocked on the reciprocal chain.
                last = q == NQB - 1
                atT2 = attp.tile([128, QB], BF16, tag="at", name="atT2")
                dsbs = []
                avus = []
                for h in range(HPC):
                    dsb = nrm.tile([HD + 1, QB], BF16, tag=f"rs{h}", name="dsb")
                    if last and h == 0:
                        nc.scalar.copy(dsb[HD:HD + 1, :],
                                       avps[h][HD:HD + 1, :])
                    else:
                        nc.vector.tensor_copy(dsb[HD:HD + 1, :],
                                              avps[h][HD:HD + 1, :])
                    dsbs.append(dsb)
                    if last:
                        # no next q-block to unblock: read avps directly and
                        # skip the extra copy hop to shorten the tail chain
                        avus.append(avps[h][0:HD, :])
                        continue
                    avu = attp.tile([HD, QB], F32, tag=f"au{h}", name="avu")
                    if h == 0:
                        nc.scalar.copy(avu, avps[h][0:HD, :])
                    else:
                        nc.vector.tensor_copy(avu, avps[h][0:HD, :])
                    avus.append(avu)
                for h in range(HPC):
                    rbps = psSt.tile([HD, QB], F32, tag="op", bufs=1,
                                     name="rbps")
                    nc.tensor.matmul(rbps, ones_f[HD:HD + 1, :],
                                     dsbs[h][HD:HD + 1, :], start=True,
                                     stop=True)
                    rrb = nrm.tile([HD, QB], F32, tag="rr", name="rrb")
                    nc.vector.reciprocal_approx_fast(out=rrb, in_=rbps)
                    if h == 0:
                        nc.vector.tensor_mul(atT2[0:HD, :], avus[h], rrb)
                    else:
                        atmp = attp.tile([HD, QB], BF16, tag="atm", name="atmp")
                        nc.vector.tensor_mul(atmp, avus[h], rrb)
                        nc.sync.dma_start(out=atT2[HD:128, :], in_=atmp)
                pending = (q, atT2)
            for lc in range(QB // 128):
                emit_outproj_chunk(*pending, lc, tail=True)
    nc.finalize()
    return nc


def _fold(M):
    # [1024, X] -> [128, 8, X]: row 128f+p lands at [p, f, :]
    X = M.shape[1]
    return np.ascontiguousarray(
        M.reshape(8, 128, X).transpose(1, 0, 2).reshape(128, 8 * X))


def _fold_blocks(M, nb):
    # [1024, L] -> [128, nb, 8, L//nb]: element (128f+p, B*b+j) lands at
    # [p, b, f, j] so each n-block load is one contiguous DRAM range.
    X = M.shape[1]
    B = X // nb
    return np.ascontiguousarray(
        M.reshape(8, 128, nb, B).transpose(1, 2, 0, 3).reshape(128, 8 * X))


def _prep_inputs(x, Wq, bq, Wk, bk, Wv, bv, Wo, bo):
    bf = ml_dtypes.bfloat16
    xT = np.ascontiguousarray(np.asarray(x, dtype=np.float32)[0].T).astype(bf)
    Wq = np.asarray(Wq, dtype=np.float32)
    Wk = np.asarray(Wk, dtype=np.float32)
    Wv = np.asarray(Wv, dtype=np.float32)
    Wo = np.asarray(Wo, dtype=np.float32)
    bq = np.asarray(bq, dtype=np.float32)
    bk = np.asarray(bk, dtype=np.float32)
    bv = np.asarray(bv, dtype=np.float32)
    has_bias = bool(np.any(bq) or np.any(bk) or np.any(bv))
    xT_f = _fold_blocks(xT, 8)
    in_maps = []
    for c in range(NCORES):
        qsl = slice(HPC * HD * c, HPC * HD * (c + 1))
        kv = c // 2
        ksl = slice(HD * kv, HD * (kv + 1))
        in_maps.append({
            "xT": xT_f,
            "wq": _fold(np.ascontiguousarray(Wq[:, qsl]).astype(bf)),
            "wk": _fold(np.ascontiguousarray(Wk[:, ksl]).astype(bf)),
            "wv": _fold(np.ascontiguousarray(Wv[:, ksl]).astype(bf)),
            "wo2": np.ascontiguousarray(
                Wo[HPC * HD * c:HPC * HD * (c + 1), :]).astype(bf),
            "bq": bq[qsl].reshape(1, -1).astype(bf),
            "bk": bk[ksl].reshape(1, -1).astype(bf),
            "bv": bv[ksl].reshape(1, -1).astype(bf),
        })
    return in_maps, has_bias


def run(inputs, trace=False):
    in_maps, has_bias = _prep_inputs(**inputs)
    key = ("nc", has_bias)
    if key not in _CACHE:
        _CACHE[key] = _build(has_bias)
    nc = _CACHE[key]
    res = run_bass_kernel_spmd(nc, in_maps, list(range(NCORES)), trace=trace)
    bo = np.asarray(inputs["bo"], dtype=np.float32)
    acc = np.zeros((L, D), dtype=np.float32)
    for r in res.results:
        acc += np.asarray(r["out"], dtype=np.float32)
    out = (acc + bo).reshape(1, L, D)
    return out, res


def kernel(**inputs):
    out, _ = run(inputs, trace=False)
    return out



# revision 18
# speedup vs baseline: 1.2102x; 1.0244x over previous
"""Grouped-Query Attention (B=1, L=4096, D=1024, 16 q-heads, 4 kv-heads, hd=64)
on 8 Trainium2 NeuronCores.

Sharding: core c owns q-heads {2c, 2c+1} and their shared kv-head c//2.
Each core computes Q/K/V projections for its heads from the full (replicated)
x, runs dense softmax attention for its 2 heads, and produces a partial
output projection  attn_heads @ Wo[head_rows]  of full shape [4096, 1024]
in bf16. Host sums the 8 partials (fp32) and adds bo.

Key engine-level structure (per core):
  - K^T projection col-packed two sequence-blocks at a time (tile_position
    col groups; separate PSUM banks per accumulation chain). V projection
    x-tile-stationary into natural [k, hd] layout, ones column appended for
    the softmax denominator (AV matmul stationary [V | 1], M=65).
  - Scores: k-tiles alternate PE row groups 0-63 / 64-127 (contraction is
    hd=64) so one group's LDWEIGHTS overlaps the other group's streaming;
    the two q-heads share each K^T stationary. Q^T is kept in both stack
    orders ([Q0;Q1] and [Q1;Q0], swapped by a GpSimd SBUF->SBUF DMA) so
    each head's moving operand exists on both partition halves.
  - Scores land in fp32 PSUM scaled by 1/16: exp runs per (group, head),
    head 0 on ScalarE (activation Exp, scale=16), head 1 mostly on VectorE
    via the custom 8-stage DVE op EXP16_ANT (minimax quadratic ^16); AV
    matmuls run one exp-group behind the score matmuls.
  - Softmax denominators ride the AV matmul (ones column, PSUM row 64);
    normalization via bf16 rank-1 PE broadcast + RECIPROCAL_APPROX_FAST.
  - Out-projection: single C=128 matmul per tile (both heads), head-1
    attention rows stacked onto partitions 64-127 by an SBUF->SBUF DMA;
    chunks spread through the next q-block's k-loop on a dedicated PSUM tag.
"""

import os

os.environ.setdefault("MYCRO_LOCAL_CACHE", "1")

import numpy as np
import ml_dtypes

import concourse.bass as bass
import concourse.bacc as bacc
import concourse.mybir as mybir
from concourse.tile import TileContext
from concourse.bass_utils import run_bass_kernel_spmd

# ---- custom DVE op: EXP16_ANT -------------------------------------------
# out = (c0 + y*(c1 + y*c2))^16 ~= exp(16*y); minimax quadratic fit of e^y
# on |y| <= 0.16 followed by 4 squarings -- exactly the 8 ALU stages of the
# v3 DVE pipeline. Registered via the documented extension path (append to
# concourse.dve_ops.OPS); the per-NEFF uop table is generated at compile.
import concourse.dve_ops as _dve_ops
from concourse.dve_ops import DveOp as _DveOp, OPS as _OPS
from concourse.dve_spec import (
    C0 as _C0, C1 as _C1, C2 as _C2, Spec as _Spec, Src0 as _Src0,
    sq as _sq, lower as _spec_lower,
)
from concourse.dve_uop import DveOpSpec as _DveOpSpec

EXP_C0 = 1.000020401766253
EXP_C1 = 1.0031922899642334
EXP_C2 = 0.4991996999298621


def _exp16_ref(in0, in1, s0, s1, imm2):
    p = (s0 + in0.astype(np.float32) * (s1 + in0.astype(np.float32) * imm2)
         ).astype(np.float32)
    for _ in range(4):
        p = (p * p).astype(np.float32)
    return p


def _register_exp16() -> "_DveOp":
    name = "EXP16_ANT"
    for op in _OPS:
        if op.name == name:
            return op
    spec = _Spec(
        body=_sq(_sq(_sq(_sq(_C0 + _Src0 * (_C1 + _Src0 * _C2))))),
        reference=_exp16_ref,
    )
    shas = {}
    for ver in ("v3", "v4"):
        try:
            shas[ver] = _DveOpSpec(
                name=name, opcode=0, uops=_spec_lower(spec, ver=ver),
                rd1_en=False).sha(ver)
        except Exception:
            pass
    op = _DveOp(name, spec, subdim=False, uops_sha=shas)
    _OPS.append(op)
    _dve_ops.CUSTOM_DVE_SPECS[name] = spec
    _dve_ops._SUB_OPCODE_FOR_NAME[name] = (
        _dve_ops._CUSTOM_DVE_ROW_BASE + len(_OPS) - 1)
    assert _dve_ops._SUB_OPCODE_FOR_NAME[name] < 0x20
    return op


EXP16_ANT = _register_exp16()
# --------------------------------------------------------------------------

BF16 = mybir.dt.bfloat16
F32 = mybir.dt.float32
AF = mybir.ActivationFunctionType

D = 1024
L = 4096
NHEAD = 16
NKV = 4
HD = 64
NCORES = 8
HPC = 2                # q heads per core
QB = 512               # q-block width
NQB = L // QB          # 8
KT = 128               # k-tile
NKT = L // KT          # 32
KG = 2                 # k-tiles per exp group
NF = D // 128          # 8 feature chunks
SCALE = 1.0 / (8.0 * 16.0)   # 1/sqrt(64) folded with the exp 1/16 pre-scale
# head-1 exp engine per group index: D=VectorE custom op, A=ScalarE
# (head-0 exp always runs on ScalarE)
EXP_PATTERN = "DDDADDDA"

_CACHE = {}


def _build(has_bias):
    nc = bacc.Bacc("TRN2", target_bir_lowering=False, debug=False)

    # folded layouts: dim0 = sbuf partition, dim1 = f-chunk * inner
    xT = nc.declare_dram_parameter("xT", [128, NF * L], BF16, isOutput=False)
    wq = nc.declare_dram_parameter("wq", [128, NF * HPC * HD], BF16,
                                   isOutput=False)
    wk = nc.declare_dram_parameter("wk", [128, NF * HD], BF16, isOutput=False)
    wv = nc.declare_dram_parameter("wv", [128, NF * HD], BF16, isOutput=False)
    wo2 = nc.declare_dram_parameter("wo2", [HPC * HD, D], BF16, isOutput=False)
    bq = nc.declare_dram_parameter("bq", [1, HPC * HD], BF16, isOutput=False)
    bk = nc.declare_dram_parameter("bk", [1, HD], BF16, isOutput=False)
    bv = nc.declare_dram_parameter("bv", [1, HD], BF16, isOutput=False)
    out = nc.declare_dram_parameter("out", [L, D], BF16, isOutput=True)

    with TileContext(nc) as tc:
        with (
            tc.tile_pool(name="sing", bufs=1) as sing,
            tc.tile_pool(name="ptp", bufs=5) as ptp,
            tc.tile_pool(name="attp", bufs=2) as attp,
            tc.tile_pool(name="nrm", bufs=2) as nrm,
            tc.tile_pool(name="obp", bufs=3) as obp,
            tc.tile_pool(name="psSt", bufs=2, space="PSUM") as psSt,
            tc.tile_pool(name="psAv", bufs=2, space="PSUM") as psAv,
        ):
            # ---- resident SBUF tensors ----
            xT_sb = sing.tile([128, NF, L], BF16)
            wq_sb = sing.tile([128, NF, HPC * HD], BF16)
            wk_sb = sing.tile([128, NF, HD], BF16)
            wv_sb = sing.tile([128, NF, HD], BF16)
            wo2_sb = sing.tile([128, D], BF16)
            KT2 = sing.tile([128, L], BF16)       # K^T duplicated on both halves
            QT2 = sing.tile([128, L], BF16)       # [Q0^T; Q1^T] * SCALE
            QT2s = sing.tile([128, L], BF16)      # swapped: [Q1^T; Q0^T]
            V_sb = sing.tile([128, NKT, HD + 1], BF16)  # col 64 = 1.0
            ones_f = sing.tile([HD + 1, HD], BF16)     # recip bcast stationary
            if has_bias:
                bq_sb = sing.tile([1, HPC * HD], BF16)
                bk_sb = sing.tile([1, HD], BF16)
                bv_sb = sing.tile([1, HD], BF16)
                ones_b = sing.tile([1, QB], BF16)

            # ---- input DMAs. All host tensors are pre-folded to
            # partition-major [128, f, *] layouts so each load is a single
            # strided-AP dma_start: the sequencers issue ~2us per
            # descriptor, so descriptor COUNT (not bytes) was the startup
            # bottleneck. First the K-projection's working set (wk + the
            # first xT n-block in two halves, split across queues), then
            # the rest.
            nc.sync.dma_start(out=wk_sb[:, :, :], in_=wk[:, :])
            engs = [nc.sync, nc.scalar]
            for b in range(NQB):
                ns = slice(QB * b, QB * (b + 1))
                for hh in range(2):
                    fs = slice(NF // 2 * hh, NF // 2 * (hh + 1))
                    nsf = slice(NF * QB * b + NF // 2 * QB * hh,
                                NF * QB * b + NF // 2 * QB * (hh + 1))
                    engs[(2 * b + hh) % 2].dma_start(out=xT_sb[:, fs, ns],
                                                     in_=xT[:, nsf])
            nc.gpsimd.dma_start(out=wv_sb[:, :, :], in_=wv[:, :])
            nc.gpsimd.dma_start(out=wq_sb[:, :, :], in_=wq[:, :])
            if has_bias:
                nc.gpsimd.dma_start(out=bq_sb, in_=bq[:, :])
                nc.gpsimd.dma_start(out=bk_sb, in_=bk[:, :])
                nc.gpsimd.dma_start(out=bv_sb, in_=bv[:, :])
                nc.gpsimd.memset(ones_b, 1.0)
            nc.gpsimd.dma_start(out=wo2_sb, in_=wo2[:, :])
            nc.gpsimd.memset(ones_f, 1.0)
            nc.gpsimd.memset(V_sb[:, :, HD], 1.0)

            # ---- projection phase, ordered to match DMA arrival of the
            # xT n-blocks: per block-pair p, the K^T chain (col-packed into
            # two PSUM banks), its KT2 dup DMAs, the V tiles covering the
            # same k range, and (p==0) the first Q projection. This keeps
            # the PE fed instead of head-of-line blocking on later blocks'
            # DMAs. ----
            def emit_kchain(p):
                nsA = slice(QB * 2 * p, QB * (2 * p + 1))
                nsB = slice(QB * (2 * p + 1), QB * (2 * p + 2))
                pp = psSt.tile([128, 2, QB], F32, tag="st", name="kps")
                for f in range(NF):
                    last = f == NF - 1 and not has_bias
                    nc.tensor.matmul(pp[0:HD, 0, :], wk_sb[:, f, :],
                                     xT_sb[:, f, nsA], start=(f == 0),
                                     stop=last, tile_position=(0, 0))
                    nc.tensor.matmul(pp[HD:128, 1, :], wk_sb[:, f, :],
                                     xT_sb[:, f, nsB], start=(f == 0),
                                     stop=last, tile_position=(0, 64))
                if has_bias:
                    nc.tensor.matmul(pp[0:HD, 0, :], bk_sb, ones_b,
                                     start=False, stop=True,
                                     tile_position=(0, 0))
                    nc.tensor.matmul(pp[HD:128, 1, :], bk_sb, ones_b,
                                     start=False, stop=True,
                                     tile_position=(0, 64))
                nc.vector.tensor_copy(KT2[0:HD, nsA], pp[0:HD, 0, :])
                nc.vector.tensor_copy(KT2[HD:128, nsB], pp[HD:128, 1, :])
                nc.scalar.dma_start(out=KT2[HD:128, nsA], in_=KT2[0:HD, nsA])
                nc.scalar.dma_start(out=KT2[0:HD, nsB], in_=KT2[HD:128, nsB])

            def emit_vtile(lt):
                ls = slice(KT * lt, KT * (lt + 1))
                vps = psSt.tile([128, HD], F32, tag="st", name="vps")
                for f in range(NF):
                    nc.tensor.matmul(vps, xT_sb[:, f, ls], wv_sb[:, f, :],
                                     start=(f == 0),
                                     stop=(not has_bias and f == NF - 1))
                if has_bias:
                    nc.tensor.matmul(vps, ones_b[:, 0:KT], bv_sb,
                                     start=False, stop=True)
                nc.vector.tensor_copy(V_sb[:, lt, 0:HD], vps)

            for p in range(NQB // 2):
                emit_kchain(p)
                for lt in range(8 * p, 8 * p + 8):
                    emit_vtile(lt)

            # ---- q-block loop (out-proj of q-1 spread inside q's k-loop) ----
            pending = None

            def emit_outproj_chunk(q, atT2, lc, tail=False):
                lcs = slice(128 * lc, 128 * (lc + 1))
                if tail:
                    # the scores 'st' slots are idle at the tail; using them
                    # double-buffers the final chunks (the single 'op' slot
                    # would serialize each chunk behind the previous DMA)
                    ops = psSt.tile([128, 2, QB], F32, tag="st", name="ops_t")
                else:
                    ops = psSt.tile([128, 2, QB], F32, tag="op", bufs=1,
                                    name="ops")
                for n in range(2):
                    ns = slice(QB * n, QB * (n + 1))
                    mm = nc.tensor.matmul(ops[:, n, :], atT2[:, lcs],
                                          wo2_sb[:, ns], start=True, stop=True)
                    if n == 1:
                        mm.ins.ldweights = False
                osb = obp.tile([128, D], BF16, tag="ob", name="osb")
                if lc % 2 == 0:
                    nc.vector.tensor_copy(osb, ops)
                else:
                    nc.scalar.copy(osb, ops)
                nc.sync.dma_start(
                    out=out[QB * q + 128 * lc:QB * q + 128 * (lc + 1), :],
                    in_=osb)

            def emit_qproj(q):
                # Q^T projection: M=128 -> [Q0^T; Q1^T] stacked, then scale
                # (scale-copy on ScalarE so the DVE exp queue can't delay it)
                qs = slice(QB * q, QB * (q + 1))
                qps = psSt.tile([128, QB], F32, tag="st", name="qps")
                for f in range(NF):
                    nc.tensor.matmul(qps, wq_sb[:, f, :], xT_sb[:, f, qs],
                                     start=(f == 0),
                                     stop=(not has_bias and f == NF - 1))
                if has_bias:
                    nc.tensor.matmul(qps, bq_sb, ones_b, start=False, stop=True)
                nc.scalar.activation(QT2[:, qs], qps, AF.Copy, scale=SCALE)
                nc.gpsimd.dma_start(out=QT2s[HD:128, qs], in_=QT2[0:HD, qs])
                nc.gpsimd.dma_start(out=QT2s[0:HD, qs], in_=QT2[HD:128, qs])

            emit_qproj(0)
            for q in range(NQB):
                qs = slice(QB * q, QB * (q + 1))
                avps = [psAv.tile([HD + 1, QB], F32, tag="av", name=f"avps{h}")
                        for h in range(HPC)]

                def emit_av(k0, gs, ptsb):
                    for j in range(gs):
                        for h in range(HPC):
                            mm = nc.tensor.matmul(avps[h], V_sb[:, k0 + j, :],
                                                  ptsb[h][:, j, :],
                                                  start=(k0 + j == 0),
                                                  stop=(k0 + j == NKT - 1))
                            if h == 1:
                                mm.ins.ldweights = False

                prevs = []  # AV runs two exp-groups behind the scores
                k = 0
                g = 0
                while k < NKT:
                    gs = min(KG, NKT - k)
                    stps = [psSt.tile([128, KG, QB], F32, tag="st",
                                      name=f"stps{h}") for h in range(HPC)]
                    ptsb = [ptp.tile([128, KG, QB], BF16, tag=f"pt{h}",
                                     name=f"ptsb{h}") for h in range(HPC)]
                    for j in range(gs):
                        kt = k + j
                        ks = slice(KT * kt, KT * (kt + 1))
                        # k-tiles alternate PE row groups; the two heads share
                        # the K^T stationary (second matmul skips LDWEIGHTS),
                        # so group A streams while group B loads and vice
                        # versa. Q0/Q1 come from the natural and swapped
                        # stacks so each lands on the right partitions.
                        if kt % 2 == 0:
                            m0 = nc.tensor.matmul(
                                stps[0][:, j, :], KT2[0:HD, ks],
                                QT2[0:HD, qs], start=True, stop=True)
                            m1 = nc.tensor.matmul(
                                stps[1][:, j, :], KT2[0:HD, ks],
                                QT2s[0:HD, qs], start=True, stop=True)
                        else:
                            m0 = nc.tensor.matmul(
                                stps[0][:, j, :], KT2[HD:128, ks],
                                QT2s[HD:128, qs], start=True, stop=True)
                            m1 = nc.tensor.matmul(
                                stps[1][:, j, :], KT2[HD:128, ks],
                                QT2[HD:128, qs], start=True, stop=True)
                        m1.ins.ldweights = False
                    nc.scalar.activation(ptsb[0][:, 0:gs, :],
                                         stps[0][:, 0:gs, :], AF.Exp,
                                         scale=16.0)
                    if EXP_PATTERN[g % len(EXP_PATTERN)] == "D":
                        nc.vector._custom_dve(
                            EXP16_ANT, out=ptsb[1][:, 0:gs, :],
                            in0=stps[1][:, 0:gs, :],
                            s0=EXP_C0, s1=EXP_C1, imm2=EXP_C2)
                    else:
                        nc.scalar.activation(ptsb[1][:, 0:gs, :],
                                             stps[1][:, 0:gs, :], AF.Exp,
                                             scale=16.0)
                    if len(prevs) >= 2:
                        emit_av(*prevs.pop(0))
                    prevs.append((k, gs, ptsb))
                    k += gs
                    g += 1
                    if pending is not None and g % 2 == 0 and 4 <= g <= 10:
                        emit_outproj_chunk(*pending, g // 2 - 2)
                        if g == 10:
                            pending = None
                    if g == 9 and q + 1 < NQB:
                        # hoisted Q projection: the qps->scale->swap-DMA chain
                        # completes during groups 10-15 so the next q-block's
                        # scores are never blocked on Q^T availability.
                        emit_qproj(q + 1)
                for pv in prevs:
                    emit_av(*pv)

                # epilogue: free avps ASAP (early PSUM->SBUF f32 copies, one
                # per engine) so the next q-block's AV chain isn't blocked on
                # the normalization chain; rank-1 denominator broadcast into
                # an 'op'-tagged PSUM slot so the 'st' rotation (scores) is
                # never blocked on the reciprocal chain.
                last = q == NQB - 1
                atT2 = attp.tile([128, QB], BF16, tag="at", name="atT2")
                dsbs = []
                avus = []
                for h in range(HPC):
                    dsb = nrm.tile([HD + 1, QB], BF16, tag=f"rs{h}", name="dsb")
                    if last and h == 0:
                        nc.scalar.copy(dsb[HD:HD + 1, :],
                                       avps[h][HD:HD + 1, :])
                    else:
                        nc.vector.tensor_copy(dsb[HD:HD + 1, :],
                                              avps[h][HD:HD + 1, :])
                    dsbs.append(dsb)
                    if last:
                        # no next q-block to unblock: read avps directly and
                        # skip the extra copy hop to shorten the tail chain
                        avus.append(avps[h][0:HD, :])
                        continue
                    avu = attp.tile([HD, QB], F32, tag=f"au{h}", name="avu")
                    if h == 0:
                        nc.scalar.copy(avu, avps[h][0:HD, :])
                    else:
                        nc.vector.tensor_copy(avu, avps[h][0:HD, :])
                    avus.append(avu)
                for h in range(HPC):
                    rbps = psSt.tile([HD, QB], F32, tag="op", bufs=1,
                                     name="rbps")
                    nc.tensor.matmul(rbps, ones_f[HD:HD + 1, :],
                                     dsbs[h][HD:HD + 1, :], start=True,
                                     stop=True)
                    rrb = nrm.tile([HD, QB], F32, tag="rr", name="rrb")
                    nc.vector.reciprocal_approx_fast(out=rrb, in_=rbps)
                    if h == 0:
                        nc.vector.tensor_mul(atT2[0:HD, :], avus[h], rrb)
                    else:
                        atmp = attp.tile([HD, QB], BF16, tag="atm", name="atmp")
                        nc.vector.tensor_mul(atmp, avus[h], rrb)
                        nc.sync.dma_start(out=atT2[HD:128, :], in_=atmp)
                pending = (q, atT2)
            for lc in range(QB // 128):
                emit_outproj_chunk(*pending, lc, tail=True)
    nc.finalize()
    return nc


def _fold(M):
    # [1024, X] -> [128, 8, X]: row 128f+p lands at [p, f, :]
    X = M.shape[1]
    return np.ascontiguousarray(
        M.reshape(8, 128, X).transpose(1, 0, 2).reshape(128, 8 * X))


def _fold_blocks(M, nb):
    # [1024, L] -> [128, nb, 8, L//nb]: element (128f+p, B*b+j) lands at
    # [p, b, f, j] so each n-block load is one contiguous DRAM range.
    X = M.shape[1]
    B = X // nb
    return np.ascontiguousarray(
        M.reshape(8, 128, nb, B).transpose(1, 2, 0, 3).reshape(128, 8 * X))


def _prep_inputs(x, Wq, bq, Wk, bk, Wv, bv, Wo, bo):
    bf = ml_dtypes.bfloat16
    xT = np.ascontiguousarray(np.asarray(x, dtype=np.float32)[0].T).astype(bf)
    Wq = np.asarray(Wq, dtype=np.float32)
    Wk = np.asarray(Wk, dtype=np.float32)
    Wv = np.asarray(Wv, dtype=np.float32)
    Wo = np.asarray(Wo, dtype=np.float32)
    bq = np.asarray(bq, dtype=np.float32)
    bk = np.asarray(bk, dtype=np.float32)
    bv = np.asarray(bv, dtype=np.float32)
    has_bias = bool(np.any(bq) or np.any(bk) or np.any(bv))
    xT_f = _fold_blocks(xT, 8)
    in_maps = []
    for c in range(NCORES):
        qsl = slice(HPC * HD * c, HPC * HD * (c + 1))
        kv = c // 2
        ksl = slice(HD * kv, HD * (kv + 1))
        in_maps.append({
            "xT": xT_f,
            "wq": _fold(np.ascontiguousarray(Wq[:, qsl]).astype(bf)),
            "wk": _fold(np.ascontiguousarray(Wk[:, ksl]).astype(bf)),
            "wv": _fold(np.ascontiguousarray(Wv[:, ksl]).astype(bf)),
            "wo2": np.ascontiguousarray(
                Wo[HPC * HD * c:HPC * HD * (c + 1), :]).astype(bf),
            "bq": bq[qsl].reshape(1, -1).astype(bf),
            "bk": bk[ksl].reshape(1, -1).astype(bf),
            "bv": bv[ksl].reshape(1, -1).astype(bf),
        })
    return in_maps, has_bias


def run(inputs, trace=False):
    in_maps, has_bias = _prep_inputs(**inputs)
    key = ("nc", has_bias)
    if key not in _CACHE:
        _CACHE[key] = _build(has_bias)
    nc = _CACHE[key]
    res = run_bass_kernel_spmd(nc, in_maps, list(range(NCORES)), trace=trace)
    bo = np.asarray(inputs["bo"], dtype=np.float32)
    acc = np.zeros((L, D), dtype=np.float32)
    for r in res.results:
        acc += np.asarray(r["out"], dtype=np.float32)
    out = (acc + bo).reshape(1, L, D)
    return out, res


def kernel(**inputs):
    out, _ = run(inputs, trace=False)
    return out



# revision 19
# speedup vs baseline: 1.2109x; 1.0005x over previous
"""Grouped-Query Attention (B=1, L=4096, D=1024, 16 q-heads, 4 kv-heads, hd=64)
on 8 Trainium2 NeuronCores.

Sharding: core c owns q-heads {2c, 2c+1} and their shared kv-head c//2.
Each core computes Q/K/V projections for its heads from the full (replicated)
x, runs dense softmax attention for its 2 heads, and produces a partial
output projection  attn_heads @ Wo[head_rows]  of full shape [4096, 1024]
in bf16. Host sums the 8 partials (fp32) and adds bo.

Key engine-level structure (per core):
  - K^T projection col-packed two sequence-blocks at a time (tile_position
    col groups; separate PSUM banks per accumulation chain). V projection
    x-tile-stationary into natural [k, hd] layout, ones column appended for
    the softmax denominator (AV matmul stationary [V | 1], M=65).
  - Scores: k-tiles alternate PE row groups 0-63 / 64-127 (contraction is
    hd=64) so one group's LDWEIGHTS overlaps the other group's streaming;
    the two q-heads share each K^T stationary. Q^T is kept in both stack
    orders ([Q0;Q1] and [Q1;Q0], swapped by a GpSimd SBUF->SBUF DMA) so
    each head's moving operand exists on both partition halves.
  - Scores land in fp32 PSUM scaled by 1/16: exp runs per (group, head),
    head 0 on ScalarE (activation Exp, scale=16), head 1 mostly on VectorE
    via the custom 8-stage DVE op EXP16_ANT (minimax quadratic ^16); AV
    matmuls run one exp-group behind the score matmuls.
  - Softmax denominators ride the AV matmul (ones column, PSUM row 64);
    normalization via bf16 rank-1 PE broadcast + RECIPROCAL_APPROX_FAST.
  - Out-projection: single C=128 matmul per tile (both heads), head-1
    attention rows stacked onto partitions 64-127 by an SBUF->SBUF DMA;
    chunks spread through the next q-block's k-loop on a dedicated PSUM tag.
"""

import os

os.environ.setdefault("MYCRO_LOCAL_CACHE", "1")

import numpy as np
import ml_dtypes

import concourse.bass as bass
import concourse.bacc as bacc
import concourse.mybir as mybir
from concourse.tile import TileContext
from concourse.bass_utils import run_bass_kernel_spmd

# ---- custom DVE op: EXP16_ANT -------------------------------------------
# out = (c0 + y*(c1 + y*c2))^16 ~= exp(16*y); minimax quadratic fit of e^y
# on |y| <= 0.16 followed by 4 squarings -- exactly the 8 ALU stages of the
# v3 DVE pipeline. Registered via the documented extension path (append to
# concourse.dve_ops.OPS); the per-NEFF uop table is generated at compile.
import concourse.dve_ops as _dve_ops
from concourse.dve_ops import DveOp as _DveOp, OPS as _OPS
from concourse.dve_spec import (
    C0 as _C0, C1 as _C1, C2 as _C2, Spec as _Spec, Src0 as _Src0,
    sq as _sq, lower as _spec_lower,
)
from concourse.dve_uop import DveOpSpec as _DveOpSpec

EXP_C0 = 1.000020401766253
EXP_C1 = 1.0031922899642334
EXP_C2 = 0.4991996999298621


def _exp16_ref(in0, in1, s0, s1, imm2):
    p = (s0 + in0.astype(np.float32) * (s1 + in0.astype(np.float32) * imm2)
         ).astype(np.float32)
    for _ in range(4):
        p = (p * p).astype(np.float32)
    return p


def _register_exp16() -> "_DveOp":
    name = "EXP16_ANT"
    for op in _OPS:
        if op.name == name:
            return op
    spec = _Spec(
        body=_sq(_sq(_sq(_sq(_C0 + _Src0 * (_C1 + _Src0 * _C2))))),
        reference=_exp16_ref,
    )
    shas = {}
    for ver in ("v3", "v4"):
        try:
            shas[ver] = _DveOpSpec(
                name=name, opcode=0, uops=_spec_lower(spec, ver=ver),
                rd1_en=False).sha(ver)
        except Exception:
            pass
    op = _DveOp(name, spec, subdim=False, uops_sha=shas)
    _OPS.append(op)
    _dve_ops.CUSTOM_DVE_SPECS[name] = spec
    _dve_ops._SUB_OPCODE_FOR_NAME[name] = (
        _dve_ops._CUSTOM_DVE_ROW_BASE + len(_OPS) - 1)
    assert _dve_ops._SUB_OPCODE_FOR_NAME[name] < 0x20
    return op


EXP16_ANT = _register_exp16()
# --------------------------------------------------------------------------

BF16 = mybir.dt.bfloat16
FP8 = mybir.dt.float8e4
F32 = mybir.dt.float32
FP8_AV = True          # exp probs in fp8e4: halves the AV moving fetch
AF = mybir.ActivationFunctionType

D = 1024
L = 4096
NHEAD = 16
NKV = 4
HD = 64
NCORES = 8
HPC = 2                # q heads per core
QB = 512               # q-block width
NQB = L // QB          # 8
KT = 128               # k-tile
NKT = L // KT          # 32
KG = 2                 # k-tiles per exp group
NF = D // 128          # 8 feature chunks
SCALE = 1.0 / (8.0 * 16.0)   # 1/sqrt(64) folded with the exp 1/16 pre-scale
# head-1 exp engine per group index: D=VectorE custom op, A=ScalarE
# (head-0 exp always runs on ScalarE)
EXP_PATTERN = "DDDADDDA"

_CACHE = {}


def _build(has_bias):
    nc = bacc.Bacc("TRN2", target_bir_lowering=False, debug=False)

    # folded layouts: dim0 = sbuf partition, dim1 = f-chunk * inner
    xT = nc.declare_dram_parameter("xT", [128, NF * L], BF16, isOutput=False)
    wq = nc.declare_dram_parameter("wq", [128, NF * HPC * HD], BF16,
                                   isOutput=False)
    wk = nc.declare_dram_parameter("wk", [128, NF * HD], BF16, isOutput=False)
    wv = nc.declare_dram_parameter("wv", [128, NF * HD], BF16, isOutput=False)
    wo2 = nc.declare_dram_parameter("wo2", [HPC * HD, D], BF16, isOutput=False)
    bq = nc.declare_dram_parameter("bq", [1, HPC * HD], BF16, isOutput=False)
    bk = nc.declare_dram_parameter("bk", [1, HD], BF16, isOutput=False)
    bv = nc.declare_dram_parameter("bv", [1, HD], BF16, isOutput=False)
    out = nc.declare_dram_parameter("out", [L, D], BF16, isOutput=True)

    with TileContext(nc) as tc:
        with (
            tc.tile_pool(name="sing", bufs=1) as sing,
            tc.tile_pool(name="ptp", bufs=5) as ptp,
            tc.tile_pool(name="attp", bufs=2) as attp,
            tc.tile_pool(name="nrm", bufs=2) as nrm,
            tc.tile_pool(name="obp", bufs=3) as obp,
            tc.tile_pool(name="psSt", bufs=2, space="PSUM") as psSt,
            tc.tile_pool(name="psAv", bufs=2, space="PSUM") as psAv,
        ):
            # ---- resident SBUF tensors ----
            xT_sb = sing.tile([128, NF, L], BF16)
            wq_sb = sing.tile([128, NF, HPC * HD], BF16)
            wk_sb = sing.tile([128, NF, HD], BF16)
            wv_sb = sing.tile([128, NF, HD], BF16)
            wo2_sb = sing.tile([128, D], BF16)
            KT2 = sing.tile([128, L], BF16)       # K^T duplicated on both halves
            QT2 = sing.tile([128, L], BF16)       # [Q0^T; Q1^T] * SCALE
            QT2s = sing.tile([128, L], BF16)      # swapped: [Q1^T; Q0^T]
            V_sb = sing.tile([128, NKT, HD + 1], BF16)  # col 64 = 1.0
            ones_f = sing.tile([HD + 1, HD], BF16)     # recip bcast stationary
            if has_bias:
                bq_sb = sing.tile([1, HPC * HD], BF16)
                bk_sb = sing.tile([1, HD], BF16)
                bv_sb = sing.tile([1, HD], BF16)
                ones_b = sing.tile([1, QB], BF16)

            # ---- input DMAs. All host tensors are pre-folded to
            # partition-major [128, f, *] layouts so each load is a single
            # strided-AP dma_start: the sequencers issue ~2us per
            # descriptor, so descriptor COUNT (not bytes) was the startup
            # bottleneck. First the K-projection's working set (wk + the
            # first xT n-block in two halves, split across queues), then
            # the rest.
            nc.sync.dma_start(out=wk_sb[:, :, :], in_=wk[:, :])
            engs = [nc.sync, nc.scalar]
            for b in range(NQB):
                ns = slice(QB * b, QB * (b + 1))
                for hh in range(2):
                    fs = slice(NF // 2 * hh, NF // 2 * (hh + 1))
                    nsf = slice(NF * QB * b + NF // 2 * QB * hh,
                                NF * QB * b + NF // 2 * QB * (hh + 1))
                    engs[(2 * b + hh) % 2].dma_start(out=xT_sb[:, fs, ns],
                                                     in_=xT[:, nsf])
            nc.gpsimd.dma_start(out=wv_sb[:, :, :], in_=wv[:, :])
            nc.gpsimd.dma_start(out=wq_sb[:, :, :], in_=wq[:, :])
            if has_bias:
                nc.gpsimd.dma_start(out=bq_sb, in_=bq[:, :])
                nc.gpsimd.dma_start(out=bk_sb, in_=bk[:, :])
                nc.gpsimd.dma_start(out=bv_sb, in_=bv[:, :])
                nc.gpsimd.memset(ones_b, 1.0)
            nc.gpsimd.dma_start(out=wo2_sb, in_=wo2[:, :])
            nc.gpsimd.memset(ones_f, 1.0)
            nc.gpsimd.memset(V_sb[:, :, HD], 1.0)

            # ---- projection phase, ordered to match DMA arrival of the
            # xT n-blocks: per block-pair p, the K^T chain (col-packed into
            # two PSUM banks), its KT2 dup DMAs, the V tiles covering the
            # same k range, and (p==0) the first Q projection. This keeps
            # the PE fed instead of head-of-line blocking on later blocks'
            # DMAs. ----
            def emit_kchain(p):
                nsA = slice(QB * 2 * p, QB * (2 * p + 1))
                nsB = slice(QB * (2 * p + 1), QB * (2 * p + 2))
                pp = psSt.tile([128, 2, QB], F32, tag="st", name="kps")
                for f in range(NF):
                    last = f == NF - 1 and not has_bias
                    nc.tensor.matmul(pp[0:HD, 0, :], wk_sb[:, f, :],
                                     xT_sb[:, f, nsA], start=(f == 0),
                                     stop=last, tile_position=(0, 0))
                    nc.tensor.matmul(pp[HD:128, 1, :], wk_sb[:, f, :],
                                     xT_sb[:, f, nsB], start=(f == 0),
                                     stop=last, tile_position=(0, 64))
                if has_bias:
                    nc.tensor.matmul(pp[0:HD, 0, :], bk_sb, ones_b,
                                     start=False, stop=True,
                                     tile_position=(0, 0))
                    nc.tensor.matmul(pp[HD:128, 1, :], bk_sb, ones_b,
                                     start=False, stop=True,
                                     tile_position=(0, 64))
                nc.vector.tensor_copy(KT2[0:HD, nsA], pp[0:HD, 0, :])
                nc.vector.tensor_copy(KT2[HD:128, nsB], pp[HD:128, 1, :])
                nc.scalar.dma_start(out=KT2[HD:128, nsA], in_=KT2[0:HD, nsA])
                nc.scalar.dma_start(out=KT2[0:HD, nsB], in_=KT2[HD:128, nsB])

            def emit_vtile(lt):
                ls = slice(KT * lt, KT * (lt + 1))
                vps = psSt.tile([128, HD], F32, tag="st", name="vps")
                for f in range(NF):
                    nc.tensor.matmul(vps, xT_sb[:, f, ls], wv_sb[:, f, :],
                                     start=(f == 0),
                                     stop=(not has_bias and f == NF - 1))
                if has_bias:
                    nc.tensor.matmul(vps, ones_b[:, 0:KT], bv_sb,
                                     start=False, stop=True)
                nc.vector.tensor_copy(V_sb[:, lt, 0:HD], vps)

            for p in range(NQB // 2):
                emit_kchain(p)
                for lt in range(8 * p, 8 * p + 8):
                    emit_vtile(lt)

            # ---- q-block loop (out-proj of q-1 spread inside q's k-loop) ----
            pending = None

            def emit_outproj_chunk(q, atT2, lc, tail=False):
                lcs = slice(128 * lc, 128 * (lc + 1))
                if tail:
                    # the scores 'st' slots are idle at the tail; using them
                    # double-buffers the final chunks (the single 'op' slot
                    # would serialize each chunk behind the previous DMA)
                    ops = psSt.tile([128, 2, QB], F32, tag="st", name="ops_t")
                else:
                    ops = psSt.tile([128, 2, QB], F32, tag="op", bufs=1,
                                    name="ops")
                for n in range(2):
                    ns = slice(QB * n, QB * (n + 1))
                    mm = nc.tensor.matmul(ops[:, n, :], atT2[:, lcs],
                                          wo2_sb[:, ns], start=True, stop=True)
                    if n == 1:
                        mm.ins.ldweights = False
                osb = obp.tile([128, D], BF16, tag="ob", name="osb")
                if lc % 2 == 0:
                    nc.vector.tensor_copy(osb, ops)
                else:
                    nc.scalar.copy(osb, ops)
                nc.sync.dma_start(
                    out=out[QB * q + 128 * lc:QB * q + 128 * (lc + 1), :],
                    in_=osb)

            def emit_qproj(q):
                # Q^T projection: M=128 -> [Q0^T; Q1^T] stacked, then scale
                # (scale-copy on ScalarE so the DVE exp queue can't delay it)
                qs = slice(QB * q, QB * (q + 1))
                qps = psSt.tile([128, QB], F32, tag="st", name="qps")
                for f in range(NF):
                    nc.tensor.matmul(qps, wq_sb[:, f, :], xT_sb[:, f, qs],
                                     start=(f == 0),
                                     stop=(not has_bias and f == NF - 1))
                if has_bias:
                    nc.tensor.matmul(qps, bq_sb, ones_b, start=False, stop=True)
                nc.scalar.activation(QT2[:, qs], qps, AF.Copy, scale=SCALE)
                nc.gpsimd.dma_start(out=QT2s[HD:128, qs], in_=QT2[0:HD, qs])
                nc.gpsimd.dma_start(out=QT2s[0:HD, qs], in_=QT2[HD:128, qs])

            emit_qproj(0)
            for q in range(NQB):
                qs = slice(QB * q, QB * (q + 1))
                avps = [psAv.tile([HD + 1, QB], F32, tag="av", name=f"avps{h}")
                        for h in range(HPC)]

                def emit_av(k0, gs, ptsb):
                    for j in range(gs):
                        for h in range(HPC):
                            mm = nc.tensor.matmul(avps[h], V_sb[:, k0 + j, :],
                                                  ptsb[h][:, j, :],
                                                  start=(k0 + j == 0),
                                                  stop=(k0 + j == NKT - 1))
                            if h == 1:
                                mm.ins.ldweights = False

                prevs = []  # AV runs two exp-groups behind the scores
                k = 0
                g = 0
                while k < NKT:
                    gs = min(KG, NKT - k)
                    stps = [psSt.tile([128, KG, QB], F32, tag="st",
                                      name=f"stps{h}") for h in range(HPC)]
                    ptsb = [ptp.tile([128, KG, QB], FP8 if FP8_AV else BF16,
                                     tag=f"pt{h}",
                                     name=f"ptsb{h}") for h in range(HPC)]
                    for j in range(gs):
                        kt = k + j
                        ks = slice(KT * kt, KT * (kt + 1))
                        # k-tiles alternate PE row groups; the two heads share
                        # the K^T stationary (second matmul skips LDWEIGHTS),
                        # so group A streams while group B loads and vice
                        # versa. Q0/Q1 come from the natural and swapped
                        # stacks so each lands on the right partitions.
                        if kt % 2 == 0:
                            m0 = nc.tensor.matmul(
                                stps[0][:, j, :], KT2[0:HD, ks],
                                QT2[0:HD, qs], start=True, stop=True)
                            m1 = nc.tensor.matmul(
                                stps[1][:, j, :], KT2[0:HD, ks],
                                QT2s[0:HD, qs], start=True, stop=True)
                        else:
                            m0 = nc.tensor.matmul(
                                stps[0][:, j, :], KT2[HD:128, ks],
                                QT2s[HD:128, qs], start=True, stop=True)
                            m1 = nc.tensor.matmul(
                                stps[1][:, j, :], KT2[HD:128, ks],
                                QT2[HD:128, qs], start=True, stop=True)
                        m1.ins.ldweights = False
                    nc.scalar.activation(ptsb[0][:, 0:gs, :],
                                         stps[0][:, 0:gs, :], AF.Exp,
                                         scale=16.0)
                    if EXP_PATTERN[g % len(EXP_PATTERN)] == "D":
                        nc.vector._custom_dve(
                            EXP16_ANT, out=ptsb[1][:, 0:gs, :],
                            in0=stps[1][:, 0:gs, :],
                            s0=EXP_C0, s1=EXP_C1, imm2=EXP_C2)
                    else:
                        nc.scalar.activation(ptsb[1][:, 0:gs, :],
                                             stps[1][:, 0:gs, :], AF.Exp,
                                             scale=16.0)
                    if len(prevs) >= 2:
                        emit_av(*prevs.pop(0))
                    prevs.append((k, gs, ptsb))
                    k += gs
                    g += 1
                    if pending is not None and g % 2 == 0 and 4 <= g <= 10:
                        emit_outproj_chunk(*pending, g // 2 - 2)
                        if g == 10:
                            pending = None
                    if g == 9 and q + 1 < NQB:
                        # hoisted Q projection: the qps->scale->swap-DMA chain
                        # completes during groups 10-15 so the next q-block's
                        # scores are never blocked on Q^T availability.
                        emit_qproj(q + 1)
                for pv in prevs:
                    emit_av(*pv)

                # epilogue: free avps ASAP (early PSUM->SBUF f32 copies, one
                # per engine) so the next q-block's AV chain isn't blocked on
                # the normalization chain; rank-1 denominator broadcast into
                # an 'op'-tagged PSUM slot so the 'st' rotation (scores) is
                # never blocked on the reciprocal chain.
                last = q == NQB - 1
                atT2 = attp.tile([128, QB], BF16, tag="at", name="atT2")
                dsbs = []
                avus = []
                for h in range(HPC):
                    dsb = nrm.tile([HD + 1, QB], BF16, tag=f"rs{h}", name="dsb")
                    if last and h == 0:
                        nc.scalar.copy(dsb[HD:HD + 1, :],
                                       avps[h][HD:HD + 1, :])
                    else:
                        nc.vector.tensor_copy(dsb[HD:HD + 1, :],
                                              avps[h][HD:HD + 1, :])
                    dsbs.append(dsb)
                    if last:
                        # no next q-block to unblock: read avps directly and
                        # skip the extra copy hop to shorten the tail chain
                        avus.append(avps[h][0:HD, :])
                        continue
                    avu = attp.tile([HD, QB], F32, tag=f"au{h}", name="avu")
                    if h == 0:
                        nc.scalar.copy(avu, avps[h][0:HD, :])
                    else:
                        nc.vector.tensor_copy(avu, avps[h][0:HD, :])
                    avus.append(avu)
                for h in range(HPC):
                    rbps = psSt.tile([HD, QB], F32, tag="op", bufs=1,
                                     name="rbps")
                    nc.tensor.matmul(rbps, ones_f[HD:HD + 1, :],
                                     dsbs[h][HD:HD + 1, :], start=True,
                                     stop=True)
                    rrb = nrm.tile([HD, QB], F32, tag="rr", name="rrb")
                    nc.vector.reciprocal_approx_fast(out=rrb, in_=rbps)
                    if h == 0:
                        nc.vector.tensor_mul(atT2[0:HD, :], avus[h], rrb)
                    else:
                        atmp = attp.tile([HD, QB], BF16, tag="atm", name="atmp")
                        nc.vector.tensor_mul(atmp, avus[h], rrb)
                        nc.sync.dma_start(out=atT2[HD:128, :], in_=atmp)
                pending = (q, atT2)
            for lc in range(QB // 128):
                emit_outproj_chunk(*pending, lc, tail=True)
    nc.finalize()
    return nc


def _fold(M):
    # [1024, X] -> [128, 8, X]: row 128f+p lands at [p, f, :]
    X = M.shape[1]
    return np.ascontiguousarray(
        M.reshape(8, 128, X).transpose(1, 0, 2).reshape(128, 8 * X))


def _fold_blocks(M, nb):
    # [1024, L] -> [128, nb, 8, L//nb]: element (128f+p, B*b+j) lands at
    # [p, b, f, j] so each n-block load is one contiguous DRAM range.
    X = M.shape[1]
    B = X // nb
    return np.ascontiguousarray(
        M.reshape(8, 128, nb, B).transpose(1, 2, 0, 3).reshape(128, 8 * X))


def _prep_inputs(x, Wq, bq, Wk, bk, Wv, bv, Wo, bo):
    bf = ml_dtypes.bfloat16
    xT = np.ascontiguousarray(np.asarray(x, dtype=np.float32)[0].T).astype(bf)
    Wq = np.asarray(Wq, dtype=np.float32)
    Wk = np.asarray(Wk, dtype=np.float32)
    Wv = np.asarray(Wv, dtype=np.float32)
    Wo = np.asarray(Wo, dtype=np.float32)
    bq = np.asarray(bq, dtype=np.float32)
    bk = np.asarray(bk, dtype=np.float32)
    bv = np.asarray(bv, dtype=np.float32)
    has_bias = bool(np.any(bq) or np.any(bk) or np.any(bv))
    xT_f = _fold_blocks(xT, 8)
    in_maps = []
    for c in range(NCORES):
        qsl = slice(HPC * HD * c, HPC * HD * (c + 1))
        kv = c // 2
        ksl = slice(HD * kv, HD * (kv + 1))
        in_maps.append({
            "xT": xT_f,
            "wq": _fold(np.ascontiguousarray(Wq[:, qsl]).astype(bf)),
            "wk": _fold(np.ascontiguousarray(Wk[:, ksl]).astype(bf)),
            "wv": _fold(np.ascontiguousarray(Wv[:, ksl]).astype(bf)),
            "wo2": np.ascontiguousarray(
                Wo[HPC * HD * c:HPC * HD * (c + 1), :]).astype(bf),
            "bq": bq[qsl].reshape(1, -1).astype(bf),
            "bk": bk[ksl].reshape(1, -1).astype(bf),
            "bv": bv[ksl].reshape(1, -1).astype(bf),
        })
    return in_maps, has_bias


def run(inputs, trace=False):
    in_maps, has_bias = _prep_inputs(**inputs)
    key = ("nc", has_bias)
    if key not in _CACHE:
        _CACHE[key] = _build(has_bias)
    nc = _CACHE[key]
    res = run_bass_kernel_spmd(nc, in_maps, list(range(NCORES)), trace=trace)
    bo = np.asarray(inputs["bo"], dtype=np.float32)
    acc = np.zeros((L, D), dtype=np.float32)
    for r in res.results:
        acc += np.asarray(r["out"], dtype=np.float32)
    out = (acc + bo).reshape(1, L, D)
    return out, res


def kernel(**inputs):
    out, _ = run(inputs, trace=False)
    return out

